# revision 26
# baseline (speedup 1.0000x reference)
"""Multi-head attention (B=4, T=2048, D=1024, H=16) on 8 TRN2 NeuronCores.

Sharding: core c = (batch b = c//2, head-group g = c%2). Each core computes
QKV projections for its 8 heads and attention; per-(tq,hp) AllGather chunks
(128KB bf16) exchange attention outputs pairwise; each core then computes
one 512-column half of the output projection for its batch (core parity
picks the half), and the host concatenates.

v5 schedule (PE-bound, ACT floor 293.5us):
  - Slim prefix: xt DMA'd in T-quarters (all j chunks of quarter 0 first)
    so V-projection starts ~5us in; prefix computes only V(tcc0..5) +
    K(oc0,tt0) + Q(oc0,tq0), everything else drains inside units.
  - attnV is deferred by 8 kc (4 slot-pairs): unit u's attnV for group g
    runs at slot g+4, spilling into unit u+1's slots 0..3. This frees
    unit 0's early slots to drain the remaining V pieces while the ACT
    exp stream starts almost immediately.
  - Per slot: scores pair (2 kc, each 2 row-grouped 64x128 matmuls packed
    into array halves) + exp per kc [128,1024] from PSUM + lagged attnV
    pair + up to 2 drain pieces (V/K/Q proj, out-proj) placed by a
    deadline-driven greedy.
  - Tail is dependency-driven (no wall-clock waits): last unit's attnV
    groups 4..7 run right after its scores; out-proj for quarter 3 is
    split into a pre-AG phase (hp0..2) that overlaps the final AllGather
    and a post-AG phase (hp3).
  - V carries 64 all-ones columns so attn@V rows 64..127 are the softmax
    denominator (partition-aligned normalize).
  - PSUM: 2x[128,1024] scores + pa attnV-accum pairs + [128,512] general
    (proj/outproj) = 8 banks.
"""

import numpy as np
import ml_dtypes

import concourse.bass as bass
import concourse.tile as tile
from concourse import mybir
from concourse.bass_utils import run_bass_kernel_spmd

BF16 = mybir.dt.bfloat16
F32 = mybir.dt.float32
NPBF16 = ml_dtypes.bfloat16

N_CORES = 8
B, T, D, H = 4, 2048, 1024, 16
DK = D // H          # 64
HL = H // 2          # heads per core (8)
NHP = HL // 2        # head pairs per core (4)
NJP = D // 128       # input-dim 128-chunks (8)
NOC = (D // 2) // 128  # per-core qkv out-dim 128-chunks (4)
NTT = T // 512       # T 512-tiles (4)
NTC = T // 128       # T 128-chunks (16)

NSLOT = 8            # kc-pair slots per unit
GLAG = 4             # unit-0 attnV group lag in slots (V pieces drain there)
LAG1 = 1             # attnV lag for units >= 1 (normalize lands early)
PREFIX_V = 0         # V pieces computed before units start (all drain)
N_WARM_MM = 46       # junk matmuls that keep HAM warm through the DMA head


def _attn_slot_of(u, g):
    """Flat slot where unit u's attnV group g is emitted."""
    return (g + GLAG) if u == 0 else (u * NSLOT + g + LAG1)

# unit order (hp, tq): staggered so each quarter's last unit lands >=2
# units before that quarter's out-proj drains, and hp first-use spreads
# the K-projection deadlines. Quarter completion indices: q0@6 q1@9
# q2@12 q3@15.
UNITS = [(0, 0), (0, 1), (1, 0), (1, 1), (2, 0), (0, 2), (3, 0), (2, 1),
         (1, 2), (3, 1), (0, 3), (2, 2), (3, 2), (2, 3), (1, 3), (3, 3)]

_uid = [0]


def _split_sync_commands(nc, max_waits=1, max_updates=1):
    """This walrus build allows only one sync wait/update command on
    sequencer-only (TPB_CTRL) instructions like Drain/NoOp; Tile's kernel
    tail drain carries one wait per logical processor. Split the excess onto
    adjacent same-engine NoOps (program order makes this equivalent)."""
    for func in nc.m.functions:
        for block in func.blocks:
            out = []
            changed = False
            for inst in block.instructions:
                si = inst.sync_info
                if si is None:
                    out.append(inst)
                    continue
                is_dma = "DMA" in type(inst).__name__.upper() or "DMA" in str(
                    getattr(inst, "opcode", "")).upper()
                waits = list(si.on_wait or [])
                # DMA completion increments must stay on the DMA instruction;
                # waits can always move to a preceding same-engine NoOp.
                updates = list(si.on_update or [])
                if is_dma:
                    updates_keep, updates = updates, []
                else:
                    updates_keep = None
                pre, post = [], []
                while len(waits) > max_waits:
                    chunk, waits = waits[:max_waits], waits[max_waits:]
                    _uid[0] += 1
                    pre.append(mybir.InstNoOp(
                        name=f"I-syncsplit-{_uid[0]}", engine=inst.engine,
                        bass_nofuse=True,
                        sync_info=mybir.SyncInfo(on_wait=chunk, on_update=[])))
                while len(updates) > max_updates:
                    chunk, updates = updates[:max_updates], updates[max_updates:]
                    _uid[0] += 1
                    post.append(mybir.InstNoOp(
                        name=f"I-syncsplit-{_uid[0]}", engine=inst.engine,
                        bass_nofuse=True,
                        sync_info=mybir.SyncInfo(on_wait=[], on_update=chunk)))
                if updates_keep is not None:
                    updates = updates_keep
                if pre or post:
                    inst.sync_info = mybir.SyncInfo(on_wait=waits, on_update=updates)
                    changed = True
                out.extend(pre)
                out.append(inst)
                out.extend(post)
            if changed:
                block.instructions = out


def _drain_schedule():
    """Deadline-driven greedy over (unit, slot) grid. Returns
    sched[u][s] = list of piece keys. Pieces:
      ("v", tcc)      V projection piece (8 MMs + bias + memset)
      ("k", oc, tt)   K projection piece
      ("q", oc, tt)   Q projection piece
      ("o", q, tcc)   out-proj piece for quarter q (q <= 2 here)
    Quarter-3 out-proj is handled in the tail.
    """
    idx_of = {u: i for i, u in enumerate(UNITS)}
    first_idx_of_hp = {hp: min(i for i, (h, _) in enumerate(UNITS) if h == hp)
                       for hp in range(NHP)}
    first_idx_of_tq = {tq: {hp: idx_of[(hp, tq)] for hp in range(NHP)}
                       for tq in range(NTT)}
    # attnV+normalize+AG for unit i complete ~half way through unit i+1;
    # quarter q's AG data is safe one unit after its last unit.
    done_idx_of_q = {q: max(i for i, (_, t) in enumerate(UNITS) if t == q) + 1
                     for q in range(NTT)}

    GRID = len(UNITS) * NSLOT   # flat slot index = u * NSLOT + s

    # xt arrives per T-quarter (DMA order: q0 ~12us, q1 ~18, q2 ~21, q3
    # ~24, wk/wq oc1-3 ~28); a drain emitted before its inputs land
    # head-of-line blocks the in-order PE queue, so every piece gets an
    # avail floor matching its DMA arrival slot.
    xt_avail = {0: 0, 1: 1, 2: 2, 3: 4}

    # Deadline pieces (latest-fit): (latest_slot, avail_slot, key).
    late_items = []
    # V pieces: vh[tcc] consumed by attnV group g=tcc//2 of unit0, emitted
    # at flat slot g+GLAG. 2-slot margin (matmuls + DVE bias/memset).
    for tcc in range(PREFIX_V, NTC):
        late_items.append(((tcc // 2) + GLAG - 2, 1, ("v", tcc)))
    # K pieces: kt[oc][:, tt] only gates scores kc=4tt (slot 2tt) of the
    # first unit with hp==oc, so later tt spill INTO that unit with a
    # 2-slot margin.
    for tt in range(NTT):
        if tt > 0:
            late_items.append((2 * tt - 2, xt_avail[tt], ("k", 0, tt)))
        for oc in range(1, NOC):
            late_items.append((first_idx_of_hp[oc] * NSLOT + 2 * tt - 2,
                               NSLOT, ("k", oc, tt)))
    # Q pieces: qt[oc][:, tq] needed at start of unit (oc, tq).
    for tq in range(NTT):
        for oc in range(NOC):
            if oc == 0 and tq == 0:
                continue  # prefix
            av = xt_avail[tq] if oc == 0 else NSLOT
            late_items.append((idx_of[(oc, tq)] * NSLOT - 1, av,
                               ("q", oc, tq)))

    # unit0 slots before GLAG carry no attnV yet -> capacity 2; all other
    # slots 1 (ACT-pace slack fits ~0.5 pieces; spread thin).
    cap = [2 if i < GLAG else 1 for i in range(GRID)]
    sched = [[] for _ in range(GRID)]

    late_items.sort(key=lambda it: it[0])   # earliest deadline first
    for dl, av, key in late_items:
        dl = min(dl, GRID - 1)
        placed = False
        for i in range(dl, min(av, dl) - 1, -1):   # latest-fit, >= avail
            if len(sched[i]) < cap[i]:
                sched[i].append(key)
                placed = True
                break
        if not placed:
            sched[max(dl, av)].append(key)  # overfill
    # out-proj pieces for quarters 0..2: every-other-slot once the
    # quarter's AG readbacks are safe. With LAG1, normalize+AG of unit i
    # start at slot 0 of unit i+1 and the readback lands ~10us (~5
    # slots) later; stride 2 so no single unit overloads.
    for q in range(NTT - 1):
        av = min((done_idx_of_q[q]) * NSLOT + 5, GRID - 1)
        pos = av
        for tcc in range(q * 4, (q + 1) * 4):
            while pos < GRID and len(sched[pos]) >= cap[pos]:
                pos += 1
            if pos < GRID:
                sched[pos].append(("o", q, tcc))
                pos += 2
            else:
                sched[GRID - 1].append(("o", q, tcc))
    return [sched[u * NSLOT:(u + 1) * NSLOT] for u in range(len(UNITS))]


def build_nc(split_sync=True):
    nc = bass.Bass("TRN2", target_bir_lowering=False, debug=False,
                   num_devices=N_CORES)

    xt_ext = nc.dram_tensor("xt", [NJP, 128, T], BF16, kind="ExternalInput").ap()
    wq_ext = nc.dram_tensor("wq", [NOC, 128, NJP, 128], BF16, kind="ExternalInput").ap()
    wk_ext = nc.dram_tensor("wk", [NOC, 128, NJP, 128], BF16, kind="ExternalInput").ap()
    wv_ext = nc.dram_tensor("wv", [128, NJP, 512], BF16, kind="ExternalInput").ap()
    wo_ext = nc.dram_tensor("wo", [128, 2, NHP, 512], BF16, kind="ExternalInput").ap()
    bq_ext = nc.dram_tensor("bq", [128, NOC], F32, kind="ExternalInput").ap()
    bk_ext = nc.dram_tensor("bk", [128, NOC], F32, kind="ExternalInput").ap()
    bv_ext = nc.dram_tensor("bv", [1, 512], F32, kind="ExternalInput").ap()
    bo_ext = nc.dram_tensor("bo", [1, 512], F32, kind="ExternalInput").ap()
    out_ext = nc.dram_tensor("out", [T, 512], F32, kind="ExternalOutput").ap()

    with tile.TileContext(nc) as tc:
        with (
            tc.tile_pool(name="persist", bufs=1) as persist,
            tc.tile_pool(name="epool", bufs=12) as epool,
            tc.tile_pool(name="evac", bufs=2) as evac,
            tc.tile_pool(name="outstage", bufs=2) as outstage,
            tc.tile_pool(name="ps", bufs=2, space="PSUM") as ps_pool,
            tc.tile_pool(name="pa", bufs=2, space="PSUM") as pa_pool,
            tc.tile_pool(name="pg", bufs=2, space="PSUM") as pg_pool,
            tc.tile_pool(name="dram", bufs=1, space="DRAM") as dram,
        ):
            # ---- weights / biases / x in, ordered for earliest V-proj ----
            wv_sb = persist.tile([128, NJP, 512], BF16, tag="wv", name="wv")
            wq_sb = persist.tile([128, NOC, NJP, 128], BF16, tag="wq", name="wq")
            wk_sb = persist.tile([128, NOC, NJP, 128], BF16, tag="wk", name="wk")
            wo_sb = persist.tile([128, 2, NHP, 512], BF16, tag="wo", name="wo")
            bq_sb = persist.tile([128, NOC], F32, tag="bq", name="bq")
            bk_sb = persist.tile([128, NOC], F32, tag="bk", name="bk")
            bv_sb = persist.tile([128, 512], F32, tag="bv", name="bv")
            bo_sb = persist.tile([128, 512], F32, tag="bo", name="bo")
            xt_sb = [persist.tile([128, T], BF16, tag=f"xt{p}", name=f"xt{p}")
                     for p in range(NJP)]

            # DMA order drives the critical path: oc0 K/Q weights + x
            # quarter 0 first (earliest scores), then Wv (V drains), the
            # remaining x quarters, then everything else.
            nc.sync.dma_start(out=bk_sb[:], in_=bk_ext[:])
            nc.sync.dma_start(out=bq_sb[:], in_=bq_ext[:])
            nc.sync.dma_start(out=wk_sb[:, 0, :, :], in_=wk_ext[0])
            nc.sync.dma_start(out=wq_sb[:, 0, :, :], in_=wq_ext[0])
            for p in range(NJP):
                nc.sync.dma_start(out=xt_sb[p][:, 0:512],
                                  in_=xt_ext[p][:, 0:512])
            nc.sync.dma_start(out=wv_sb[:], in_=wv_ext[:])
            nc.sync.dma_start(
                out=bv_sb[:],
                in_=bass.AP(tensor=bv_ext.tensor, offset=bv_ext.offset,
                            ap=[[0, 128]] + list(bv_ext.ap[1:])))
            for quarter in range(1, 4):
                for p in range(NJP):
                    nc.sync.dma_start(
                        out=xt_sb[p][:, quarter * 512:(quarter + 1) * 512],
                        in_=xt_ext[p][:, quarter * 512:(quarter + 1) * 512])
            for oc in range(1, NOC):
                nc.sync.dma_start(out=wk_sb[:, oc, :, :], in_=wk_ext[oc])
                nc.sync.dma_start(out=wq_sb[:, oc, :, :], in_=wq_ext[oc])
            nc.sync.dma_start(out=wo_sb[:], in_=wo_ext[:])
            nc.sync.dma_start(
                out=bo_sb[:],
                in_=bass.AP(tensor=bo_ext.tensor, offset=bo_ext.offset,
                            ap=[[0, 128]] + list(bo_ext.ap[1:])))

            qt_sb = [persist.tile([128, T], BF16, tag=f"qt{i}", name=f"qt{i}") for i in range(NOC)]
            kt_sb = [persist.tile([128, T], BF16, tag=f"kt{i}", name=f"kt{i}") for i in range(NOC)]
            vh_sb = [persist.tile([128, HL, 128], BF16, tag=f"vh{i}", name=f"vh{i}") for i in range(NTC)]
            ot_a = [persist.tile([128, NHP, 512], BF16, tag=f"ot_a{q}", name=f"ot_a{q}")
                    for q in range(NTT)]
            ot_b = [persist.tile([128, NHP, 512], BF16, tag=f"ot_b{q}", name=f"ot_b{q}")
                    for q in range(NTT)]

            # ---- emission helpers ----
            def emit_v(tcc):
                """V projection piece for one T 128-chunk via pg pool."""
                psv = pg_pool.tile([128, 512], F32, tag="pg", name="pg")
                for j in range(NJP):
                    nc.tensor.matmul(
                        psv[:],
                        lhsT=xt_sb[j][:, tcc * 128:(tcc + 1) * 128],
                        rhs=wv_sb[:, j, :],
                        start=(j == 0), stop=(j == NJP - 1))
                nc.vector.tensor_tensor(
                    vh_sb[tcc][:, :, 0:64],
                    psv[:].rearrange("p (h d) -> p h d", h=HL),
                    bv_sb[:].rearrange("p (h d) -> p h d", h=HL),
                    mybir.AluOpType.add)
                nc.vector.memset(vh_sb[tcc][:, :, 64:128], 1.0)

            def emit_kq(oc, tt, which):
                """K/Q projection piece via the pg pool."""
                w_sb, b_sb, dst = ((wk_sb, bk_sb, kt_sb) if which == "k"
                                   else (wq_sb, bq_sb, qt_sb))
                psq = pg_pool.tile([128, 512], F32, tag="pg", name="pg")
                for j in range(NJP):
                    nc.tensor.matmul(
                        psq[:],
                        lhsT=w_sb[:, oc, j, :],
                        rhs=xt_sb[j][:, tt * 512:(tt + 1) * 512],
                        start=(j == 0), stop=(j == NJP - 1))
                nc.vector.tensor_scalar_add(
                    dst[oc][:, tt * 512:(tt + 1) * 512], psq[:],
                    b_sb[:, oc:oc + 1])

            def emit_outproj_tcc(q, tcc, skip_hp=None):
                tl = tcc % 4
                pso = pg_pool.tile([128, 512], F32, tag="pg", name="pg")
                first = True
                for src_i, ot_sb in ((0, ot_a), (1, ot_b)):
                    for hp in range(NHP):
                        nc.tensor.matmul(
                            pso[:],
                            lhsT=ot_sb[q][:, hp, tl * 128:(tl + 1) * 128],
                            rhs=wo_sb[:, src_i, hp, :],
                            start=first,
                            stop=(src_i == 1 and hp == NHP - 1))
                        first = False
                ost = outstage.tile([128, 512], F32, tag="ost", name="ost")
                nc.vector.tensor_add(ost[:], pso[:], bo_sb[:])
                nc.sync.dma_start(
                    out=out_ext[tcc * 128:(tcc + 1) * 128, :],
                    in_=ost[:])

            def emit_drain(key):
                kind = key[0]
                if kind == "v":
                    emit_v(key[1])
                elif kind in ("k", "q"):
                    emit_kq(key[1], key[2], kind)
                else:
                    emit_outproj_tcc(key[1], key[2])

            # ---- warm the ACT exp table during the DMA window: walrus
            # attaches the ~2.7us ACT_TABLE_LOAD to the first ACTIVATE ----
            warm = persist.tile([128, 8], F32, tag="warm", name="warm")
            nc.vector.memset(warm[:], 0.0)
            nc.scalar.activation(warm[:], warm[:],
                                 mybir.ActivationFunctionType.Exp)
            # ---- keep HAM un-throttled through the DMA head: ~11.5us of
            # junk matmuls so the prefix K00/Q00 run at 2.4GHz, not 1.2 ----
            junk_w = persist.tile([128, 128], BF16, tag="junkw", name="junkw")
            junk_x = persist.tile([128, 512], BF16, tag="junkx", name="junkx")
            nc.vector.memset(junk_w[:], 0.0)
            nc.vector.memset(junk_x[:], 0.0)
            warm_ps = pg_pool.tile([128, 512], F32, tag="pg", name="pg")
            for _ in range(N_WARM_MM):
                nc.tensor.matmul(warm_ps[:], lhsT=junk_w[:], rhs=junk_x[:],
                                 start=True, stop=True)

            # ---- prefix: just K(oc0,tt0) + Q(oc0,tq0) so scores start
            # the moment their DMAs land; V pieces are unit-0 drains ----
            emit_kq(0, 0, "k")
            emit_kq(0, 0, "q")
            for tcc in range(PREFIX_V):
                emit_v(tcc)

            # ---- main units ----
            sched = _drain_schedule()

            po2_of_unit = {}

            def alloc_po2(ui):
                po2_of_unit[ui] = [
                    pa_pool.tile([128, 512], F32, tag="pa", name="pa")
                    for _ in range(2)]
                return po2_of_unit[ui]

            e_tiles = {}   # (ui, kc) -> sbuf exp tile

            def emit_scores(ui, kc):
                hp, tq = UNITS[ui]
                ps = ps_pool.tile([128, 1024], F32, tag="ps", name="ps")
                for h2 in (0, 1):
                    nc.tensor.matmul(
                        ps[:, h2 * 512:(h2 + 1) * 512],
                        lhsT=kt_sb[hp][h2 * 64:(h2 + 1) * 64, kc * 128:(kc + 1) * 128],
                        rhs=qt_sb[hp][h2 * 64:(h2 + 1) * 64, tq * 512:(tq + 1) * 512],
                        start=True, stop=True,
                        tile_position=(h2 * 64, 0))
                e_t = epool.tile([128, 1024], BF16, tag="e", name="e")
                nc.scalar.activation(e_t[:], ps[:],
                                     mybir.ActivationFunctionType.Exp)
                e_tiles[(ui, kc)] = e_t

            def emit_attnv(ui, kc):
                po2 = po2_of_unit[ui]
                e_t = e_tiles.pop((ui, kc))
                for h2 in (0, 1):
                    nc.tensor.matmul(
                        po2[h2][:],
                        lhsT=vh_sb[kc][:, 2 * UNITS[ui][0] + h2, :],
                        rhs=e_t[:, h2 * 512:(h2 + 1) * 512],
                        start=(kc == 0), stop=(kc == NTC - 1))

            def emit_normalize_ag(ui):
                """Evacuate attnV accumulators, normalize, exchange."""
                hp, tq = UNITS[ui]
                po2 = po2_of_unit.pop(ui)
                om = evac.tile([128, 512], F32, tag="om", name="om")
                pk = evac.tile([128, 512], F32, tag="pk", name="pk")
                for h2 in (0, 1):
                    nc.vector.tensor_copy(
                        pk[h2 * 64:(h2 + 1) * 64, :], po2[h2][64:128, :])
                    nc.vector.tensor_copy(
                        om[h2 * 64:(h2 + 1) * 64, :], po2[h2][0:64, :])
                rr = evac.tile([128, 512], F32, tag="rr", name="rr")
                # ~51 ULP is far inside the 2e-2 tolerance; denominators are
                # sums of exps (~2e3), no edge cases. 5x faster than
                # reciprocal() and off the AG critical chain sooner.
                nc.vector.reciprocal_approx_fast(rr[:], pk[:])
                nc.vector.tensor_mul(ot_a[tq][:, hp, :], om[:], rr[:])

                oT_in = dram.tile([128, 512], BF16, name=f"oT_in{tq}_{hp}")
                oT_out = dram.tile([2, 128, 512], BF16, name=f"oT_out{tq}_{hp}")
                nc.sync.dma_start(out=oT_in[:], in_=ot_a[tq][:, hp, :])
                nc.gpsimd.collective_compute(
                    "AllGather",
                    mybir.AluOpType.bypass,
                    ins=[oT_in.opt()],
                    outs=[oT_out.opt()],
                    replica_groups=[[0, 1], [2, 3], [4, 5], [6, 7]],
                )
                nc.sync.dma_start(out=ot_a[tq][:, hp, :], in_=oT_out[0])
                nc.sync.dma_start(out=ot_b[tq][:, hp, :], in_=oT_out[1])

            # flat slot -> attnV groups emitted there (unit0 lags GLAG
            # slots so V-projection drains fit; units >=1 lag LAG1 so
            # normalize+AG land early)
            attn_slot = {}
            for u in range(len(UNITS)):
                for g in range(NSLOT):
                    attn_slot.setdefault(_attn_slot_of(u, g), []).append((u, g))

            for ui in range(len(UNITS)):
                alloc_po2(ui)
                for s in range(NSLOT):
                    emit_scores(ui, 2 * s)
                    emit_scores(ui, 2 * s + 1)
                    for vu, vg in attn_slot.get(ui * NSLOT + s, []):
                        emit_attnv(vu, 2 * vg)
                        emit_attnv(vu, 2 * vg + 1)
                        if vg == NSLOT - 1:
                            emit_normalize_ag(vu)
                    for key in sched[ui][s]:
                        emit_drain(key)

            # ---- tail: last unit's spilled attnV groups, its normalize/AG,
            # then quarter-3 out-proj split around the final AllGather ----
            last = len(UNITS) - 1
            for vu, vg in [pair for f, pairs in attn_slot.items()
                           if f >= len(UNITS) * NSLOT for pair in pairs]:
                emit_attnv(vu, 2 * vg)
                emit_attnv(vu, 2 * vg + 1)
            emit_normalize_ag(last)

            # phase A: hp0..2 contributions for quarter 3 accumulate while
            # AG(q3,hp3) is in flight; phase B finishes with the hp3 matmuls.
            # hp emission follows AG readiness ((0,3) early, (2,3) then
            # (1,3) late) so the in-order PE queue never blocks.
            psos = [None] * 4
            for i, tcc in enumerate(range(3 * 4, 4 * 4)):
                pool, tag = (pg_pool, "pg") if i < 2 else (pa_pool, "pa")
                psos[i] = pool.tile([128, 512], F32, tag=tag, name=tag)
            for hi, hp in enumerate((0, 2, 1)):
                for i, tcc in enumerate(range(3 * 4, 4 * 4)):
                    tl = tcc % 4
                    for src_i, ot_sb in ((0, ot_a), (1, ot_b)):
                        nc.tensor.matmul(
                            psos[i][:],
                            lhsT=ot_sb[3][:, hp, tl * 128:(tl + 1) * 128],
                            rhs=wo_sb[:, src_i, hp, :],
                            start=(hi == 0 and src_i == 0), stop=False)
            # phase B: src-major so the second AG readback hides behind the
            # first source's four matmuls.
            for j, (src_i, ot_sb) in enumerate(((0, ot_a), (1, ot_b))):
                for i, tcc in enumerate(range(3 * 4, 4 * 4)):
                    pso, tl = psos[i], tcc % 4
                    nc.tensor.matmul(
                        pso[:],
                        lhsT=ot_sb[3][:, NHP - 1, tl * 128:(tl + 1) * 128],
                        rhs=wo_sb[:, src_i, NHP - 1, :],
                        start=False, stop=(j == 1))
            for i, tcc in enumerate(range(3 * 4, 4 * 4)):
                ost = outstage.tile([128, 512], F32, tag="ost", name="ost")
                nc.vector.tensor_add(ost[:], psos[i][:], bo_sb[:])
                nc.sync.dma_start(
                    out=out_ext[tcc * 128:(tcc + 1) * 128, :],
                    in_=ost[:])

    if split_sync:
        _split_sync_commands(nc)
    return nc


_NC_CACHE = {}


def _get_nc():
    if "nc" not in _NC_CACHE:
        _NC_CACHE["nc"] = build_nc()
    return _NC_CACHE["nc"]


def _prep_core_inputs(x, Wq, bq, Wk, bk, Wv, bv, Wo, bo):
    """Host-side sharding + layout. Returns in_maps list (8 cores)."""
    x = np.asarray(x, np.float32)
    s = 1.0 / np.sqrt(np.float32(DK))
    Wq_s, bq_s = np.asarray(Wq, np.float32) * s, np.asarray(bq, np.float32) * s
    Wk_f, bk_f = np.asarray(Wk, np.float32), np.asarray(bk, np.float32)
    Wv_f, bv_f = np.asarray(Wv, np.float32), np.asarray(bv, np.float32)
    Wo_f, bo_f = np.asarray(Wo, np.float32), np.asarray(bo, np.float32)

    in_maps = []
    for c in range(N_CORES):
        b, g = c // 2, c % 2
        cols = slice(g * 512, (g + 1) * 512)
        wq_g, bq_g = Wq_s[:, cols], bq_s[cols]
        wk_g, bk_g = Wk_f[:, cols], bk_f[cols]
        wv_g, bv_g = Wv_f[:, cols], bv_f[cols]

        xt_dev = np.ascontiguousarray(x[b].T).reshape(NJP, 128, T).astype(NPBF16)

        def wqk_dev(w):
            # [jp, r, oc, c] -> [oc, r, jp, c] (oc-major for split DMA)
            return np.ascontiguousarray(
                w.reshape(NJP, 128, NOC, 128).transpose(2, 1, 0, 3)).astype(NPBF16)

        wv_dev = np.ascontiguousarray(
            wv_g.reshape(NJP, 128, 512).transpose(1, 0, 2)).astype(NPBF16)

        # Wo rows regrouped to the on-device O^T layout, columns = this
        # core's output half (nb = core parity g):
        # rows [src group, hp, h2, 64] -> partitions h2*64+r, free [src, hp, col]
        wo_dev = (Wo_f[:, cols].reshape(2, NHP, 2, 64, 512)
                  .transpose(2, 3, 0, 1, 4)          # [h2, r, src, hp, col]
                  .reshape(128, 2, NHP, 512)).astype(NPBF16)
        bo_dev = bo_f[cols].reshape(1, 512)

        in_maps.append({
            "xt": xt_dev,
            "wq": wqk_dev(wq_g), "wk": wqk_dev(wk_g), "wv": wv_dev,
            "wo": wo_dev,
            "bq": np.ascontiguousarray(bq_g.reshape(NOC, 128).T),
            "bk": np.ascontiguousarray(bk_g.reshape(NOC, 128).T),
            "bv": bv_g.reshape(1, 512),
            "bo": bo_dev,
        })
    return in_maps


def kernel(x, Wq, bq, Wk, bk, Wv, bv, Wo, bo, _trace=False):
    nc = _get_nc()
    in_maps = _prep_core_inputs(x, Wq, bq, Wk, bk, Wv, bv, Wo, bo)
    res = run_bass_kernel_spmd(nc, in_maps, core_ids=list(range(N_CORES)),
                               trace=_trace)
    out = np.empty((B, T, D), np.float32)
    for b in range(B):
        for g in range(2):
            out[b][:, g * 512:(g + 1) * 512] = res.results[2 * b + g]["out"]
    if _trace:
        kernel.last_result = res
    return out


# revision 33
# speedup vs baseline: 1.0185x; 1.0185x over previous
"""Multi-head attention (B=4, T=2048, D=1024, H=16) on 8 TRN2 NeuronCores.

Sharding: core c = (batch b = c//2, head-group g = c%2). Each core computes
QKV projections for its 8 heads and attention; per-(tq,hp) AllGather chunks
(128KB bf16) exchange attention outputs pairwise; each core then computes
one 512-column half of the output projection for its batch (core parity
picks the half), and the host concatenates.

v5 schedule (PE-bound, ACT floor 293.5us):
  - Slim prefix: xt DMA'd in T-quarters (all j chunks of quarter 0 first)
    so V-projection starts ~5us in; prefix computes only V(tcc0..5) +
    K(oc0,tt0) + Q(oc0,tq0), everything else drains inside units.
  - attnV is deferred by 8 kc (4 slot-pairs): unit u's attnV for group g
    runs at slot g+4, spilling into unit u+1's slots 0..3. This frees
    unit 0's early slots to drain the remaining V pieces while the ACT
    exp stream starts almost immediately.
  - Per slot: scores pair (2 kc, each 2 row-grouped 64x128 matmuls packed
    into array halves) + exp per kc [128,1024] from PSUM + lagged attnV
    pair + up to 2 drain pieces (V/K/Q proj, out-proj) placed by a
    deadline-driven greedy.
  - Tail is dependency-driven (no wall-clock waits): last unit's attnV
    groups 4..7 run right after its scores; out-proj for quarter 3 is
    split into a pre-AG phase (hp0..2) that overlaps the final AllGather
    and a post-AG phase (hp3).
  - V carries 64 all-ones columns so attn@V rows 64..127 are the softmax
    denominator (partition-aligned normalize).
  - PSUM: 2x[128,1024] scores + pa attnV-accum pairs + [128,512] general
    (proj/outproj) = 8 banks.
"""

import numpy as np
import ml_dtypes

import concourse.bass as bass
import concourse.tile as tile
from concourse import mybir
from concourse.bass_utils import run_bass_kernel_spmd

BF16 = mybir.dt.bfloat16
F32 = mybir.dt.float32
NPBF16 = ml_dtypes.bfloat16

N_CORES = 8
B, T, D, H = 4, 2048, 1024, 16
DK = D // H          # 64
HL = H // 2          # heads per core (8)
NHP = HL // 2        # head pairs per core (4)
NJP = D // 128       # input-dim 128-chunks (8)
NOC = (D // 2) // 128  # per-core qkv out-dim 128-chunks (4)
NTT = T // 512       # T 512-tiles (4)
NTC = T // 128       # T 128-chunks (16)

NSLOT = 8            # kc-pair slots per unit
GLAG = 4             # unit-0 attnV group lag in slots (V pieces drain there)
LAG1 = 1             # attnV lag for units >= 1 (normalize lands early)
PREFIX_V = 0         # V pieces computed before units start (all drain)
N_WARM_MM = 14       # junk matmuls that keep HAM warm through the DMA head
                     # (engines start ~8us in; DMA for the prefix lands ~13.5)


def _attn_slot_of(u, g):
    """Flat slot where unit u's attnV group g is emitted."""
    return (g + GLAG) if u == 0 else (u * NSLOT + g + LAG1)

# unit order (hp, tq): staggered so each quarter's last unit lands >=2
# units before that quarter's out-proj drains, and hp first-use spreads
# the K-projection deadlines. Quarter completion indices: q0@6 q1@9
# q2@12 q3@15.
UNITS = [(0, 0), (0, 1), (1, 0), (1, 1), (2, 0), (0, 2), (3, 0), (2, 1),
         (1, 2), (3, 1), (0, 3), (2, 2), (3, 2), (2, 3), (1, 3), (3, 3)]

_uid = [0]


def _split_sync_commands(nc, max_waits=1, max_updates=1):
    """This walrus build allows only one sync wait/update command on
    sequencer-only (TPB_CTRL) instructions like Drain/NoOp; Tile's kernel
    tail drain carries one wait per logical processor. Split the excess onto
    adjacent same-engine NoOps (program order makes this equivalent)."""
    for func in nc.m.functions:
        for block in func.blocks:
            out = []
            changed = False
            for inst in block.instructions:
                si = inst.sync_info
                if si is None:
                    out.append(inst)
                    continue
                is_dma = "DMA" in type(inst).__name__.upper() or "DMA" in str(
                    getattr(inst, "opcode", "")).upper()
                waits = list(si.on_wait or [])
                # DMA completion increments must stay on the DMA instruction;
                # waits can always move to a preceding same-engine NoOp.
                updates = list(si.on_update or [])
                if is_dma:
                    updates_keep, updates = updates, []
                else:
                    updates_keep = None
                pre, post = [], []
                while len(waits) > max_waits:
                    chunk, waits = waits[:max_waits], waits[max_waits:]
                    _uid[0] += 1
                    pre.append(mybir.InstNoOp(
                        name=f"I-syncsplit-{_uid[0]}", engine=inst.engine,
                        bass_nofuse=True,
                        sync_info=mybir.SyncInfo(on_wait=chunk, on_update=[])))
                while len(updates) > max_updates:
                    chunk, updates = updates[:max_updates], updates[max_updates:]
                    _uid[0] += 1
                    post.append(mybir.InstNoOp(
                        name=f"I-syncsplit-{_uid[0]}", engine=inst.engine,
                        bass_nofuse=True,
                        sync_info=mybir.SyncInfo(on_wait=[], on_update=chunk)))
                if updates_keep is not None:
                    updates = updates_keep
                if pre or post:
                    inst.sync_info = mybir.SyncInfo(on_wait=waits, on_update=updates)
                    changed = True
                out.extend(pre)
                out.append(inst)
                out.extend(post)
            if changed:
                block.instructions = out


def _drain_schedule():
    """Deadline-driven greedy over (unit, slot) grid. Returns
    sched[u][s] = list of piece keys. Pieces:
      ("v", tcc)      V projection piece (8 MMs + bias + memset)
      ("k", oc, tt)   K projection piece
      ("q", oc, tt)   Q projection piece
      ("o", q, tcc)   out-proj piece for quarter q (q <= 2 here)
    Quarter-3 out-proj is handled in the tail.
    """
    idx_of = {u: i for i, u in enumerate(UNITS)}
    first_idx_of_hp = {hp: min(i for i, (h, _) in enumerate(UNITS) if h == hp)
                       for hp in range(NHP)}
    first_idx_of_tq = {tq: {hp: idx_of[(hp, tq)] for hp in range(NHP)}
                       for tq in range(NTT)}
    # attnV+normalize+AG for unit i complete ~half way through unit i+1;
    # quarter q's AG data is safe one unit after its last unit.
    done_idx_of_q = {q: max(i for i, (_, t) in enumerate(UNITS) if t == q) + 1
                     for q in range(NTT)}

    GRID = len(UNITS) * NSLOT   # flat slot index = u * NSLOT + s

    # xt arrives per T-quarter (DMA order: q0 ~12us, q1 ~18, q2 ~21, q3
    # ~24, wk/wq oc1-3 ~28); a drain emitted before its inputs land
    # head-of-line blocks the in-order PE queue, so every piece gets an
    # avail floor matching its DMA arrival slot.
    xt_avail = {0: 0, 1: 1, 2: 2, 3: 4}

    # Deadline pieces (latest-fit): (latest_slot, avail_slot, key).
    late_items = []
    # V pieces: vh[tcc] consumed by attnV group g=tcc//2 of unit0, emitted
    # at flat slot g+GLAG. 2-slot margin (matmuls + DVE bias/memset).
    for tcc in range(PREFIX_V, NTC):
        late_items.append(((tcc // 2) + GLAG - 2, 1, ("v", tcc)))
    # K pieces: kt[oc][:, tt] only gates scores kc=4tt (slot 2tt) of the
    # first unit with hp==oc, so later tt spill INTO that unit with a
    # 2-slot margin.
    for tt in range(NTT):
        if tt > 0:
            late_items.append((2 * tt - 2, xt_avail[tt], ("k", 0, tt)))
        for oc in range(1, NOC):
            late_items.append((first_idx_of_hp[oc] * NSLOT + 2 * tt - 2,
                               NSLOT, ("k", oc, tt)))
    # Q pieces: qt[oc][:, tq] needed at start of unit (oc, tq).
    for tq in range(NTT):
        for oc in range(NOC):
            if oc == 0 and tq == 0:
                continue  # prefix
            av = xt_avail[tq] if oc == 0 else NSLOT
            late_items.append((idx_of[(oc, tq)] * NSLOT - 2, av,
                               ("q", oc, tq)))

    # unit0 slots before GLAG carry no attnV yet -> capacity 2; all other
    # slots 1 (ACT-pace slack fits ~0.5 pieces; spread thin).
    cap = [2 if i < GLAG else 1 for i in range(GRID)]
    sched = [[] for _ in range(GRID)]

    late_items.sort(key=lambda it: it[0])   # earliest deadline first
    for dl, av, key in late_items:
        dl = min(dl, GRID - 1)
        placed = False
        for i in range(dl, min(av, dl) - 1, -1):   # latest-fit, >= avail
            if len(sched[i]) < cap[i]:
                sched[i].append(key)
                placed = True
                break
        if not placed:
            sched[max(dl, av)].append(key)  # overfill
    # out-proj pieces for quarters 0..2: every-other-slot once the
    # quarter's AG readbacks are safe. With LAG1, normalize+AG of unit i
    # start at slot 0 of unit i+1 and the readback lands ~10us (~5
    # slots) later; stride 2 so no single unit overloads.
    for q in range(NTT - 1):
        av = min((done_idx_of_q[q]) * NSLOT + 5, GRID - 1)
        pos = av
        for tcc in range(q * 4, (q + 1) * 4):
            while pos < GRID and len(sched[pos]) >= cap[pos]:
                pos += 1
            if pos < GRID:
                sched[pos].append(("o", q, tcc))
                pos += 2
            else:
                sched[GRID - 1].append(("o", q, tcc))
    return [sched[u * NSLOT:(u + 1) * NSLOT] for u in range(len(UNITS))]


def build_nc(split_sync=True):
    nc = bass.Bass("TRN2", target_bir_lowering=False, debug=False,
                   num_devices=N_CORES)

    xt_ext = nc.dram_tensor("xt", [NJP, 128, T], BF16, kind="ExternalInput").ap()
    wq_ext = nc.dram_tensor("wq", [NOC, 128, NJP, 128], BF16, kind="ExternalInput").ap()
    wk_ext = nc.dram_tensor("wk", [NOC, 128, NJP, 128], BF16, kind="ExternalInput").ap()
    wv_ext = nc.dram_tensor("wv", [128, NJP, 512], BF16, kind="ExternalInput").ap()
    wo_ext = nc.dram_tensor("wo", [128, 2, NHP, 512], BF16, kind="ExternalInput").ap()
    bq_ext = nc.dram_tensor("bq", [128, NOC], F32, kind="ExternalInput").ap()
    bk_ext = nc.dram_tensor("bk", [128, NOC], F32, kind="ExternalInput").ap()
    bv_ext = nc.dram_tensor("bv", [1, 512], F32, kind="ExternalInput").ap()
    bo_ext = nc.dram_tensor("bo", [1, 512], F32, kind="ExternalInput").ap()
    out_ext = nc.dram_tensor("out", [T, 512], F32, kind="ExternalOutput").ap()

    with tile.TileContext(nc) as tc:
        with (
            tc.tile_pool(name="persist", bufs=1) as persist,
            tc.tile_pool(name="epool", bufs=12) as epool,
            tc.tile_pool(name="evac", bufs=2) as evac,
            tc.tile_pool(name="outstage", bufs=2) as outstage,
            tc.tile_pool(name="ps", bufs=2, space="PSUM") as ps_pool,
            tc.tile_pool(name="pa", bufs=2, space="PSUM") as pa_pool,
            tc.tile_pool(name="pg", bufs=2, space="PSUM") as pg_pool,
            tc.tile_pool(name="dram", bufs=1, space="DRAM") as dram,
        ):
            # ---- weights / biases / x in, ordered for earliest V-proj ----
            wv_sb = persist.tile([128, NJP, 512], BF16, tag="wv", name="wv")
            wq_sb = persist.tile([128, NOC, NJP, 128], BF16, tag="wq", name="wq")
            wk_sb = persist.tile([128, NOC, NJP, 128], BF16, tag="wk", name="wk")
            wo_sb = persist.tile([128, 2, NHP, 512], BF16, tag="wo", name="wo")
            bq_sb = persist.tile([128, NOC], F32, tag="bq", name="bq")
            bk_sb = persist.tile([128, NOC], F32, tag="bk", name="bk")
            bv_sb = persist.tile([128, 512], F32, tag="bv", name="bv")
            bo_sb = persist.tile([128, 512], F32, tag="bo", name="bo")
            xt_sb = [persist.tile([128, T], BF16, tag=f"xt{p}", name=f"xt{p}")
                     for p in range(NJP)]

            # DMA order drives the critical path: oc0 K/Q weights + x
            # quarter 0 first (earliest scores), then Wv (V drains), the
            # remaining x quarters, then everything else.
            nc.sync.dma_start(out=bk_sb[:], in_=bk_ext[:])
            nc.sync.dma_start(out=bq_sb[:], in_=bq_ext[:])
            nc.sync.dma_start(out=wk_sb[:, 0, :, :], in_=wk_ext[0])
            nc.sync.dma_start(out=wq_sb[:, 0, :, :], in_=wq_ext[0])
            for p in range(NJP):
                nc.sync.dma_start(out=xt_sb[p][:, 0:512],
                                  in_=xt_ext[p][:, 0:512])
            nc.sync.dma_start(out=wv_sb[:], in_=wv_ext[:])
            nc.sync.dma_start(
                out=bv_sb[:],
                in_=bass.AP(tensor=bv_ext.tensor, offset=bv_ext.offset,
                            ap=[[0, 128]] + list(bv_ext.ap[1:])))
            for quarter in range(1, 4):
                for p in range(NJP):
                    nc.sync.dma_start(
                        out=xt_sb[p][:, quarter * 512:(quarter + 1) * 512],
                        in_=xt_ext[p][:, quarter * 512:(quarter + 1) * 512])
            for oc in range(1, NOC):
                nc.sync.dma_start(out=wk_sb[:, oc, :, :], in_=wk_ext[oc])
                nc.sync.dma_start(out=wq_sb[:, oc, :, :], in_=wq_ext[oc])
            nc.sync.dma_start(out=wo_sb[:], in_=wo_ext[:])
            nc.sync.dma_start(
                out=bo_sb[:],
                in_=bass.AP(tensor=bo_ext.tensor, offset=bo_ext.offset,
                            ap=[[0, 128]] + list(bo_ext.ap[1:])))

            qt_sb = [persist.tile([128, T], BF16, tag=f"qt{i}", name=f"qt{i}") for i in range(NOC)]
            kt_sb = [persist.tile([128, T], BF16, tag=f"kt{i}", name=f"kt{i}") for i in range(NOC)]
            vh_sb = [persist.tile([128, HL, 128], BF16, tag=f"vh{i}", name=f"vh{i}") for i in range(NTC)]
            ot_a = [persist.tile([128, NHP, 512], BF16, tag=f"ot_a{q}", name=f"ot_a{q}")
                    for q in range(NTT)]
            ot_b = [persist.tile([128, NHP, 512], BF16, tag=f"ot_b{q}", name=f"ot_b{q}")
                    for q in range(NTT)]

            # ---- emission helpers ----
            def emit_v(tcc):
                """V projection piece for one T 128-chunk via pg pool."""
                psv = pg_pool.tile([128, 512], F32, tag="pg", name="pg")
                for j in range(NJP):
                    nc.tensor.matmul(
                        psv[:],
                        lhsT=xt_sb[j][:, tcc * 128:(tcc + 1) * 128],
                        rhs=wv_sb[:, j, :],
                        start=(j == 0), stop=(j == NJP - 1))
                nc.vector.tensor_tensor(
                    vh_sb[tcc][:, :, 0:64],
                    psv[:].rearrange("p (h d) -> p h d", h=HL),
                    bv_sb[:].rearrange("p (h d) -> p h d", h=HL),
                    mybir.AluOpType.add)
                nc.vector.memset(vh_sb[tcc][:, :, 64:128], 1.0)

            def emit_kq(oc, tt, which):
                """K/Q projection piece via the pg pool."""
                w_sb, b_sb, dst = ((wk_sb, bk_sb, kt_sb) if which == "k"
                                   else (wq_sb, bq_sb, qt_sb))
                psq = pg_pool.tile([128, 512], F32, tag="pg", name="pg")
                for j in range(NJP):
                    nc.tensor.matmul(
                        psq[:],
                        lhsT=w_sb[:, oc, j, :],
                        rhs=xt_sb[j][:, tt * 512:(tt + 1) * 512],
                        start=(j == 0), stop=(j == NJP - 1))
                nc.vector.tensor_scalar_add(
                    dst[oc][:, tt * 512:(tt + 1) * 512], psq[:],
                    b_sb[:, oc:oc + 1])

            def emit_outproj_tcc(q, tcc, skip_hp=None):
                tl = tcc % 4
                pso = pg_pool.tile([128, 512], F32, tag="pg", name="pg")
                first = True
                for src_i, ot_sb in ((0, ot_a), (1, ot_b)):
                    for hp in range(NHP):
                        nc.tensor.matmul(
                            pso[:],
                            lhsT=ot_sb[q][:, hp, tl * 128:(tl + 1) * 128],
                            rhs=wo_sb[:, src_i, hp, :],
                            start=first,
                            stop=(src_i == 1 and hp == NHP - 1))
                        first = False
                ost = outstage.tile([128, 512], F32, tag="ost", name="ost")
                nc.vector.tensor_add(ost[:], pso[:], bo_sb[:])
                nc.sync.dma_start(
                    out=out_ext[tcc * 128:(tcc + 1) * 128, :],
                    in_=ost[:])

            def emit_drain(key):
                kind = key[0]
                if kind == "v":
                    emit_v(key[1])
                elif kind in ("k", "q"):
                    emit_kq(key[1], key[2], kind)
                else:
                    emit_outproj_tcc(key[1], key[2])

            # ---- warm the ACT exp table during the DMA window: walrus
            # attaches the ~2.7us ACT_TABLE_LOAD to the first ACTIVATE ----
            warm = persist.tile([128, 8], F32, tag="warm", name="warm")
            nc.vector.memset(warm[:], 0.0)
            nc.scalar.activation(warm[:], warm[:],
                                 mybir.ActivationFunctionType.Exp)
            # ---- keep HAM un-throttled through the DMA head: ~11.5us of
            # junk matmuls so the prefix K00/Q00 run at 2.4GHz, not 1.2 ----
            junk_w = persist.tile([128, 128], BF16, tag="junkw", name="junkw")
            junk_x = persist.tile([128, 512], BF16, tag="junkx", name="junkx")
            nc.vector.memset(junk_w[:], 0.0)
            nc.vector.memset(junk_x[:], 0.0)
            warm_ps = pg_pool.tile([128, 512], F32, tag="pg", name="pg")
            for _ in range(N_WARM_MM):
                nc.tensor.matmul(warm_ps[:], lhsT=junk_w[:], rhs=junk_x[:],
                                 start=True, stop=True)

            # ---- prefix: just K(oc0,tt0) + Q(oc0,tq0) so scores start
            # the moment their DMAs land; V pieces are unit-0 drains ----
            emit_kq(0, 0, "k")
            emit_kq(0, 0, "q")
            for tcc in range(PREFIX_V):
                emit_v(tcc)

            # ---- main units ----
            sched = _drain_schedule()

            po2_of_unit = {}

            def alloc_po2(ui):
                po2_of_unit[ui] = [
                    pa_pool.tile([128, 512], F32, tag="pa", name="pa")
                    for _ in range(2)]
                return po2_of_unit[ui]

            e_tiles = {}   # (ui, kc) -> sbuf exp tile

            def emit_scores(ui, kc):
                hp, tq = UNITS[ui]
                ps = ps_pool.tile([128, 1024], F32, tag="ps", name="ps")
                for h2 in (0, 1):
                    nc.tensor.matmul(
                        ps[:, h2 * 512:(h2 + 1) * 512],
                        lhsT=kt_sb[hp][h2 * 64:(h2 + 1) * 64, kc * 128:(kc + 1) * 128],
                        rhs=qt_sb[hp][h2 * 64:(h2 + 1) * 64, tq * 512:(tq + 1) * 512],
                        start=True, stop=True,
                        tile_position=(h2 * 64, 0))
                e_t = epool.tile([128, 1024], BF16, tag="e", name="e")
                nc.scalar.activation(e_t[:], ps[:],
                                     mybir.ActivationFunctionType.Exp)
                e_tiles[(ui, kc)] = e_t

            def emit_attnv(ui, kc):
                po2 = po2_of_unit[ui]
                e_t = e_tiles.pop((ui, kc))
                for h2 in (0, 1):
                    nc.tensor.matmul(
                        po2[h2][:],
                        lhsT=vh_sb[kc][:, 2 * UNITS[ui][0] + h2, :],
                        rhs=e_t[:, h2 * 512:(h2 + 1) * 512],
                        start=(kc == 0), stop=(kc == NTC - 1))

            def emit_normalize_ag(ui, halves=1):
                """Evacuate attnV accumulators, normalize (single fused
                divide), exchange. halves=2 pipelines the last unit's tail:
                each 256-column half normalizes and AllGathers separately so
                the first readback lands ~4us earlier."""
                hp, tq = UNITS[ui]
                po2 = po2_of_unit.pop(ui)
                om = evac.tile([128, 512], F32, tag="om", name="om")
                pk = evac.tile([128, 512], F32, tag="pk", name="pk")
                rr = evac.tile([128, 512], F32, tag="rr", name="rr")
                hw = 512 // halves
                for hf in range(halves):
                    sl = slice(hf * hw, (hf + 1) * hw)
                    for h2 in (0, 1):
                        nc.vector.tensor_copy(
                            pk[h2 * 64:(h2 + 1) * 64, sl], po2[h2][64:128, sl])
                        nc.vector.tensor_copy(
                            om[h2 * 64:(h2 + 1) * 64, sl], po2[h2][0:64, sl])
                    nc.vector.reciprocal(rr[:, sl], pk[:, sl])
                    nc.vector.tensor_mul(ot_a[tq][:, hp, sl], om[:, sl],
                                         rr[:, sl])
                    oT_in = dram.tile([128, hw], BF16, name=f"oTi{tq}_{hp}_{hf}")
                    oT_out = dram.tile([2, 128, hw], BF16,
                                       name=f"oTo{tq}_{hp}_{hf}")
                    nc.sync.dma_start(out=oT_in[:], in_=ot_a[tq][:, hp, sl])
                    nc.gpsimd.collective_compute(
                        "AllGather",
                        mybir.AluOpType.bypass,
                        ins=[oT_in.opt()],
                        outs=[oT_out.opt()],
                        replica_groups=[[0, 1], [2, 3], [4, 5], [6, 7]],
                    )
                    nc.sync.dma_start(out=ot_a[tq][:, hp, sl], in_=oT_out[0])
                    nc.sync.dma_start(out=ot_b[tq][:, hp, sl], in_=oT_out[1])

            # flat slot -> attnV groups emitted there (unit0 lags GLAG
            # slots so V-projection drains fit; units >=1 lag LAG1 so
            # normalize+AG land early)
            attn_slot = {}
            for u in range(len(UNITS)):
                for g in range(NSLOT):
                    attn_slot.setdefault(_attn_slot_of(u, g), []).append((u, g))

            for ui in range(len(UNITS)):
                alloc_po2(ui)
                for s in range(NSLOT):
                    emit_scores(ui, 2 * s)
                    emit_scores(ui, 2 * s + 1)
                    done_units = []
                    for vu, vg in attn_slot.get(ui * NSLOT + s, []):
                        emit_attnv(vu, 2 * vg)
                        emit_attnv(vu, 2 * vg + 1)
                        if vg == NSLOT - 1:
                            done_units.append(vu)
                    for key in sched[ui][s]:
                        emit_drain(key)
                    # normalize last: its ~6us DVE chain (copies + iterative
                    # divide) must not head-of-line block the drains'
                    # bias-adds that gate upcoming scores.
                    for vu in done_units:
                        emit_normalize_ag(vu)

            # ---- tail: last unit's spilled attnV groups, its normalize/AG,
            # then quarter-3 out-proj split around the final AllGather ----
            last = len(UNITS) - 1
            for vu, vg in [pair for f, pairs in attn_slot.items()
                           if f >= len(UNITS) * NSLOT for pair in pairs]:
                emit_attnv(vu, 2 * vg)
                emit_attnv(vu, 2 * vg + 1)
            emit_normalize_ag(last, halves=2)

            # phase A: hp0..2 contributions for quarter 3 accumulate while
            # AG(q3,hp3) is in flight; phase B finishes with the hp3 matmuls.
            # hp emission follows AG readiness ((0,3) early, (2,3) then
            # (1,3) late) so the in-order PE queue never blocks.
            psos = [None] * 4
            for i, tcc in enumerate(range(3 * 4, 4 * 4)):
                pool, tag = (pg_pool, "pg") if i < 2 else (pa_pool, "pa")
                psos[i] = pool.tile([128, 512], F32, tag=tag, name=tag)
            for hi, hp in enumerate((0, 2, 1)):
                for i, tcc in enumerate(range(3 * 4, 4 * 4)):
                    tl = tcc % 4
                    for src_i, ot_sb in ((0, ot_a), (1, ot_b)):
                        nc.tensor.matmul(
                            psos[i][:],
                            lhsT=ot_sb[3][:, hp, tl * 128:(tl + 1) * 128],
                            rhs=wo_sb[:, src_i, hp, :],
                            start=(hi == 0 and src_i == 0), stop=False)
            # phase B: half-major (matches the split AllGather halves of the
            # last chunk) so tl0/tl1 finish while half1 is still in flight.
            for hf in range(2):
                for j, (src_i, ot_sb) in enumerate(((0, ot_a), (1, ot_b))):
                    for i in (2 * hf, 2 * hf + 1):
                        tcc = 3 * 4 + i
                        pso, tl = psos[i], tcc % 4
                        nc.tensor.matmul(
                            pso[:],
                            lhsT=ot_sb[3][:, NHP - 1, tl * 128:(tl + 1) * 128],
                            rhs=wo_sb[:, src_i, NHP - 1, :],
                            start=False, stop=(j == 1))
                for i in (2 * hf, 2 * hf + 1):
                    tcc = 3 * 4 + i
                    ost = outstage.tile([128, 512], F32, tag="ost", name="ost")
                    nc.vector.tensor_add(ost[:], psos[i][:], bo_sb[:])
                    nc.sync.dma_start(
                        out=out_ext[tcc * 128:(tcc + 1) * 128, :],
                        in_=ost[:])

    if split_sync:
        _split_sync_commands(nc)
    return nc


_NC_CACHE = {}


def _get_nc():
    if "nc" not in _NC_CACHE:
        _NC_CACHE["nc"] = build_nc()
    return _NC_CACHE["nc"]


def _prep_core_inputs(x, Wq, bq, Wk, bk, Wv, bv, Wo, bo):
    """Host-side sharding + layout. Returns in_maps list (8 cores)."""
    x = np.asarray(x, np.float32)
    s = 1.0 / np.sqrt(np.float32(DK))
    Wq_s, bq_s = np.asarray(Wq, np.float32) * s, np.asarray(bq, np.float32) * s
    Wk_f, bk_f = np.asarray(Wk, np.float32), np.asarray(bk, np.float32)
    Wv_f, bv_f = np.asarray(Wv, np.float32), np.asarray(bv, np.float32)
    Wo_f, bo_f = np.asarray(Wo, np.float32), np.asarray(bo, np.float32)

    in_maps = []
    for c in range(N_CORES):
        b, g = c // 2, c % 2
        cols = slice(g * 512, (g + 1) * 512)
        wq_g, bq_g = Wq_s[:, cols], bq_s[cols]
        wk_g, bk_g = Wk_f[:, cols], bk_f[cols]
        wv_g, bv_g = Wv_f[:, cols], bv_f[cols]

        xt_dev = np.ascontiguousarray(x[b].T).reshape(NJP, 128, T).astype(NPBF16)

        def wqk_dev(w):
            # [jp, r, oc, c] -> [oc, r, jp, c] (oc-major for split DMA)
            return np.ascontiguousarray(
                w.reshape(NJP, 128, NOC, 128).transpose(2, 1, 0, 3)).astype(NPBF16)

        wv_dev = np.ascontiguousarray(
            wv_g.reshape(NJP, 128, 512).transpose(1, 0, 2)).astype(NPBF16)

        # Wo rows regrouped to the on-device O^T layout, columns = this
        # core's output half (nb = core parity g):
        # rows [src group, hp, h2, 64] -> partitions h2*64+r, free [src, hp, col]
        wo_dev = (Wo_f[:, cols].reshape(2, NHP, 2, 64, 512)
                  .transpose(2, 3, 0, 1, 4)          # [h2, r, src, hp, col]
                  .reshape(128, 2, NHP, 512)).astype(NPBF16)
        bo_dev = bo_f[cols].reshape(1, 512)

        in_maps.append({
            "xt": xt_dev,
            "wq": wqk_dev(wq_g), "wk": wqk_dev(wk_g), "wv": wv_dev,
            "wo": wo_dev,
            "bq": np.ascontiguousarray(bq_g.reshape(NOC, 128).T),
            "bk": np.ascontiguousarray(bk_g.reshape(NOC, 128).T),
            "bv": bv_g.reshape(1, 512),
            "bo": bo_dev,
        })
    return in_maps


def kernel(x, Wq, bq, Wk, bk, Wv, bv, Wo, bo, _trace=False):
    nc = _get_nc()
    in_maps = _prep_core_inputs(x, Wq, bq, Wk, bk, Wv, bv, Wo, bo)
    res = run_bass_kernel_spmd(nc, in_maps, core_ids=list(range(N_CORES)),
                               trace=_trace)
    out = np.empty((B, T, D), np.float32)
    for b in range(B):
        for g in range(2):
            out[b][:, g * 512:(g + 1) * 512] = res.results[2 * b + g]["out"]
    if _trace:
        kernel.last_result = res
    return out


# revision 39
# speedup vs baseline: 1.0315x; 1.0127x over previous
"""Multi-head attention (B=4, T=2048, D=1024, H=16) on 8 TRN2 NeuronCores.

Sharding: core c = (batch b = c//2, head-group g = c%2). Each core computes
QKV projections for its 8 heads and attention; per-(tq,hp) AllGather chunks
(128KB bf16) exchange attention outputs pairwise; each core then computes
one 512-column half of the output projection for its batch (core parity
picks the half), and the host concatenates.

v5 schedule (PE-bound, ACT floor 293.5us):
  - Slim prefix: xt DMA'd in T-quarters (all j chunks of quarter 0 first)
    so V-projection starts ~5us in; prefix computes only V(tcc0..5) +
    K(oc0,tt0) + Q(oc0,tq0), everything else drains inside units.
  - attnV is deferred by 8 kc (4 slot-pairs): unit u's attnV for group g
    runs at slot g+4, spilling into unit u+1's slots 0..3. This frees
    unit 0's early slots to drain the remaining V pieces while the ACT
    exp stream starts almost immediately.
  - Per slot: scores pair (2 kc, each 2 row-grouped 64x128 matmuls packed
    into array halves) + exp per kc [128,1024] from PSUM + lagged attnV
    pair + up to 2 drain pieces (V/K/Q proj, out-proj) placed by a
    deadline-driven greedy.
  - Tail is dependency-driven (no wall-clock waits): last unit's attnV
    groups 4..7 run right after its scores; out-proj for quarter 3 is
    split into a pre-AG phase (hp0..2) that overlaps the final AllGather
    and a post-AG phase (hp3).
  - V carries 64 all-ones columns so attn@V rows 64..127 are the softmax
    denominator (partition-aligned normalize).
  - PSUM: 2x[128,1024] scores + pa attnV-accum pairs + [128,512] general
    (proj/outproj) = 8 banks.
"""

import numpy as np
import ml_dtypes

import concourse.bass as bass
import concourse.tile as tile
from concourse import mybir
from concourse.bass_utils import run_bass_kernel_spmd

BF16 = mybir.dt.bfloat16
F32 = mybir.dt.float32
NPBF16 = ml_dtypes.bfloat16

N_CORES = 8
B, T, D, H = 4, 2048, 1024, 16
DK = D // H          # 64
HL = H // 2          # heads per core (8)
NHP = HL // 2        # head pairs per core (4)
NJP = D // 128       # input-dim 128-chunks (8)
NOC = (D // 2) // 128  # per-core qkv out-dim 128-chunks (4)
NTT = T // 512       # T 512-tiles (4)
NTC = T // 128       # T 128-chunks (16)

NSLOT = 8            # kc-pair slots per unit
GLAG = 4             # unit-0 attnV group lag in slots (V pieces drain there)
LAG1 = 1             # attnV lag for units >= 1 (normalize lands early)
PREFIX_V = 0         # V pieces computed before units start (all drain)
N_WARM_MM = 14       # junk matmuls that keep HAM warm through the DMA head
                     # (engines start ~8us in; DMA for the prefix lands ~13.5)


def _attn_slot_of(u, g):
    """Flat slot where unit u's attnV group g is emitted."""
    return (g + GLAG) if u == 0 else (u * NSLOT + g + LAG1)

# unit order (hp, tq): staggered so each quarter's last unit lands >=2
# units before that quarter's out-proj drains, and hp first-use spreads
# the K-projection deadlines. Quarter completion indices: q0@6 q1@9
# q2@12 q3@15.
UNITS = [(0, 0), (0, 1), (1, 0), (1, 1), (2, 0), (0, 2), (3, 0), (2, 1),
         (1, 2), (3, 1), (0, 3), (2, 2), (3, 2), (2, 3), (1, 3), (3, 3)]

_uid = [0]


def _split_sync_commands(nc, max_waits=1, max_updates=1):
    """This walrus build allows only one sync wait/update command on
    sequencer-only (TPB_CTRL) instructions like Drain/NoOp; Tile's kernel
    tail drain carries one wait per logical processor. Split the excess onto
    adjacent same-engine NoOps (program order makes this equivalent)."""
    for func in nc.m.functions:
        for block in func.blocks:
            out = []
            changed = False
            for inst in block.instructions:
                si = inst.sync_info
                if si is None:
                    out.append(inst)
                    continue
                is_dma = "DMA" in type(inst).__name__.upper() or "DMA" in str(
                    getattr(inst, "opcode", "")).upper()
                waits = list(si.on_wait or [])
                # DMA completion increments must stay on the DMA instruction;
                # waits can always move to a preceding same-engine NoOp.
                updates = list(si.on_update or [])
                if is_dma:
                    updates_keep, updates = updates, []
                else:
                    updates_keep = None
                pre, post = [], []
                while len(waits) > max_waits:
                    chunk, waits = waits[:max_waits], waits[max_waits:]
                    _uid[0] += 1
                    pre.append(mybir.InstNoOp(
                        name=f"I-syncsplit-{_uid[0]}", engine=inst.engine,
                        bass_nofuse=True,
                        sync_info=mybir.SyncInfo(on_wait=chunk, on_update=[])))
                while len(updates) > max_updates:
                    chunk, updates = updates[:max_updates], updates[max_updates:]
                    _uid[0] += 1
                    post.append(mybir.InstNoOp(
                        name=f"I-syncsplit-{_uid[0]}", engine=inst.engine,
                        bass_nofuse=True,
                        sync_info=mybir.SyncInfo(on_wait=[], on_update=chunk)))
                if updates_keep is not None:
                    updates = updates_keep
                if pre or post:
                    inst.sync_info = mybir.SyncInfo(on_wait=waits, on_update=updates)
                    changed = True
                out.extend(pre)
                out.append(inst)
                out.extend(post)
            if changed:
                block.instructions = out


def _drain_schedule():
    """Deadline-driven greedy over (unit, slot) grid. Returns
    sched[u][s] = list of piece keys. Pieces:
      ("v", tcc)      V projection piece (8 MMs + bias + memset)
      ("k", oc, tt)   K projection piece
      ("q", oc, tt)   Q projection piece
      ("o", q, tcc)   out-proj piece for quarter q (q <= 2 here)
    Quarter-3 out-proj is handled in the tail.
    """
    idx_of = {u: i for i, u in enumerate(UNITS)}
    first_idx_of_hp = {hp: min(i for i, (h, _) in enumerate(UNITS) if h == hp)
                       for hp in range(NHP)}
    first_idx_of_tq = {tq: {hp: idx_of[(hp, tq)] for hp in range(NHP)}
                       for tq in range(NTT)}
    # attnV+normalize+AG for unit i complete ~half way through unit i+1;
    # quarter q's AG data is safe one unit after its last unit.
    done_idx_of_q = {q: max(i for i, (_, t) in enumerate(UNITS) if t == q) + 1
                     for q in range(NTT)}

    GRID = len(UNITS) * NSLOT   # flat slot index = u * NSLOT + s

    # xt arrives per T-quarter (DMA order: q0 ~12us, q1 ~18, q2 ~21, q3
    # ~24, wk/wq oc1-3 ~28); a drain emitted before its inputs land
    # head-of-line blocks the in-order PE queue, so every piece gets an
    # avail floor matching its DMA arrival slot.
    xt_avail = {0: 0, 1: 2, 2: 3, 3: 5}

    # Deadline pieces (latest-fit): (latest_slot, avail_slot, key).
    late_items = []
    # V pieces: vh[tcc] consumed by attnV group g=tcc//2 of unit0, emitted
    # at flat slot g+GLAG. 2-slot margin (matmuls + DVE bias/memset).
    for tcc in range(PREFIX_V, NTC):
        late_items.append(((tcc // 2) + GLAG - 2, 1, ("v", tcc)))
    # K pieces: kt[oc][:, tt] only gates scores kc=4tt (slot 2tt) of the
    # first unit with hp==oc, so later tt spill INTO that unit with a
    # 2-slot margin.
    for tt in range(NTT):
        if tt > 0:
            late_items.append((2 * tt - 2, xt_avail[tt], ("k", 0, tt)))
        for oc in range(1, NOC):
            late_items.append((first_idx_of_hp[oc] * NSLOT + 2 * tt - 2,
                               NSLOT, ("k", oc, tt)))
    # Q pieces: qt[oc][:, tq] needed at start of unit (oc, tq).
    for tq in range(NTT):
        for oc in range(NOC):
            if oc == 0 and tq == 0:
                continue  # prefix
            av = xt_avail[tq] if oc == 0 else NSLOT
            late_items.append((idx_of[(oc, tq)] * NSLOT - 2, av,
                               ("q", oc, tq)))

    # unit0 slots before GLAG carry no attnV yet -> capacity 2; all other
    # slots 1 (ACT-pace slack fits ~0.5 pieces; spread thin).
    cap = [2 if i < GLAG else 1 for i in range(GRID)]
    sched = [[] for _ in range(GRID)]

    late_items.sort(key=lambda it: it[0])   # earliest deadline first
    for dl, av, key in late_items:
        dl = min(dl, GRID - 1)
        placed = False
        for i in range(dl, min(av, dl) - 1, -1):   # latest-fit, >= avail
            if len(sched[i]) < cap[i]:
                sched[i].append(key)
                placed = True
                break
        if not placed:
            sched[max(dl, av)].append(key)  # overfill
    # out-proj pieces for quarters 0..2: every-other-slot once the
    # quarter's AG readbacks are safe. With LAG1, normalize+AG of unit i
    # start at slot 0 of unit i+1 and the readback lands ~10us (~5
    # slots) later; stride 2 so no single unit overloads.
    for q in range(NTT - 1):
        av = min(done_idx_of_q[q] * NSLOT + NSLOT + 4, GRID - 1)
        pos = av
        for tcc in range(q * 4, (q + 1) * 4):
            while pos < GRID and len(sched[pos]) >= cap[pos]:
                pos += 1
            if pos < GRID:
                sched[pos].append(("o", q, tcc))
                pos += 2
            else:
                sched[GRID - 1].append(("o", q, tcc))
    return [sched[u * NSLOT:(u + 1) * NSLOT] for u in range(len(UNITS))]


def build_nc(split_sync=True):
    nc = bass.Bass("TRN2", target_bir_lowering=False, debug=False,
                   num_devices=N_CORES)

    xt_ext = nc.dram_tensor("xt", [NJP, 128, T], BF16, kind="ExternalInput").ap()
    wq_ext = nc.dram_tensor("wq", [NOC, 128, NJP, 128], BF16, kind="ExternalInput").ap()
    wk_ext = nc.dram_tensor("wk", [NOC, 128, NJP, 128], BF16, kind="ExternalInput").ap()
    wv_ext = nc.dram_tensor("wv", [128, NJP, 512], BF16, kind="ExternalInput").ap()
    wo_ext = nc.dram_tensor("wo", [128, 2, NHP, 512], BF16, kind="ExternalInput").ap()
    bq_ext = nc.dram_tensor("bq", [128, NOC], F32, kind="ExternalInput").ap()
    bk_ext = nc.dram_tensor("bk", [128, NOC], F32, kind="ExternalInput").ap()
    bv_ext = nc.dram_tensor("bv", [1, 512], F32, kind="ExternalInput").ap()
    bo_ext = nc.dram_tensor("bo", [1, 512], F32, kind="ExternalInput").ap()
    out_ext = nc.dram_tensor("out", [T, 512], F32, kind="ExternalOutput").ap()

    with tile.TileContext(nc) as tc:
        with (
            tc.tile_pool(name="persist", bufs=1) as persist,
            tc.tile_pool(name="epool", bufs=12) as epool,
            tc.tile_pool(name="evac", bufs=2) as evac,
            tc.tile_pool(name="outstage", bufs=2) as outstage,
            tc.tile_pool(name="ps", bufs=2, space="PSUM") as ps_pool,
            tc.tile_pool(name="pa", bufs=2, space="PSUM") as pa_pool,
            tc.tile_pool(name="pg", bufs=2, space="PSUM") as pg_pool,
            tc.tile_pool(name="dram", bufs=1, space="DRAM") as dram,
        ):
            # ---- weights / biases / x in, ordered for earliest V-proj ----
            wv_sb = persist.tile([128, NJP, 512], BF16, tag="wv", name="wv")
            wq_sb = persist.tile([128, NOC, NJP, 128], BF16, tag="wq", name="wq")
            wk_sb = persist.tile([128, NOC, NJP, 128], BF16, tag="wk", name="wk")
            wo_sb = persist.tile([128, 2, NHP, 512], BF16, tag="wo", name="wo")
            bq_sb = persist.tile([128, NOC], F32, tag="bq", name="bq")
            bk_sb = persist.tile([128, NOC], F32, tag="bk", name="bk")
            bv_sb = persist.tile([128, 512], F32, tag="bv", name="bv")
            bo_sb = persist.tile([128, 512], F32, tag="bo", name="bo")
            xt_sb = [persist.tile([128, T], BF16, tag=f"xt{p}", name=f"xt{p}")
                     for p in range(NJP)]

            # DMA order drives the critical path: oc0 K/Q weights + x
            # quarter 0 first (earliest scores), then Wv (V drains), the
            # remaining x quarters, then everything else.
            nc.sync.dma_start(out=bk_sb[:], in_=bk_ext[:])
            nc.sync.dma_start(out=bq_sb[:], in_=bq_ext[:])
            nc.sync.dma_start(out=wk_sb[:, 0, :, :], in_=wk_ext[0])
            nc.sync.dma_start(out=wq_sb[:, 0, :, :], in_=wq_ext[0])
            for p in range(NJP):
                nc.sync.dma_start(out=xt_sb[p][:, 0:512],
                                  in_=xt_ext[p][:, 0:512])
            nc.sync.dma_start(out=wv_sb[:], in_=wv_ext[:])
            nc.sync.dma_start(
                out=bv_sb[:],
                in_=bass.AP(tensor=bv_ext.tensor, offset=bv_ext.offset,
                            ap=[[0, 128]] + list(bv_ext.ap[1:])))
            for quarter in range(1, 4):
                for p in range(NJP):
                    nc.sync.dma_start(
                        out=xt_sb[p][:, quarter * 512:(quarter + 1) * 512],
                        in_=xt_ext[p][:, quarter * 512:(quarter + 1) * 512])
            for oc in range(1, NOC):
                nc.sync.dma_start(out=wk_sb[:, oc, :, :], in_=wk_ext[oc])
                nc.sync.dma_start(out=wq_sb[:, oc, :, :], in_=wq_ext[oc])
            nc.sync.dma_start(out=wo_sb[:], in_=wo_ext[:])
            nc.sync.dma_start(
                out=bo_sb[:],
                in_=bass.AP(tensor=bo_ext.tensor, offset=bo_ext.offset,
                            ap=[[0, 128]] + list(bo_ext.ap[1:])))

            qt_sb = [persist.tile([128, T], BF16, tag=f"qt{i}", name=f"qt{i}") for i in range(NOC)]
            kt_sb = [persist.tile([128, T], BF16, tag=f"kt{i}", name=f"kt{i}") for i in range(NOC)]
            vh_sb = [persist.tile([128, HL, 128], BF16, tag=f"vh{i}", name=f"vh{i}") for i in range(NTC)]
            ot_a = [persist.tile([128, NHP, 512], BF16, tag=f"ot_a{q}", name=f"ot_a{q}")
                    for q in range(NTT)]
            ot_b = [persist.tile([128, NHP, 512], BF16, tag=f"ot_b{q}", name=f"ot_b{q}")
                    for q in range(NTT)]

            # ---- emission helpers ----
            def emit_v(tcc):
                """V projection piece for one T 128-chunk via pg pool."""
                psv = pg_pool.tile([128, 512], F32, tag="pg", name="pg")
                for j in range(NJP):
                    nc.tensor.matmul(
                        psv[:],
                        lhsT=xt_sb[j][:, tcc * 128:(tcc + 1) * 128],
                        rhs=wv_sb[:, j, :],
                        start=(j == 0), stop=(j == NJP - 1))
                nc.vector.tensor_tensor(
                    vh_sb[tcc][:, :, 0:64],
                    psv[:].rearrange("p (h d) -> p h d", h=HL),
                    bv_sb[:].rearrange("p (h d) -> p h d", h=HL),
                    mybir.AluOpType.add)
                nc.vector.memset(vh_sb[tcc][:, :, 64:128], 1.0)

            def emit_kq(oc, tt, which):
                """K/Q projection piece via the pg pool."""
                w_sb, b_sb, dst = ((wk_sb, bk_sb, kt_sb) if which == "k"
                                   else (wq_sb, bq_sb, qt_sb))
                psq = pg_pool.tile([128, 512], F32, tag="pg", name="pg")
                for j in range(NJP):
                    nc.tensor.matmul(
                        psq[:],
                        lhsT=w_sb[:, oc, j, :],
                        rhs=xt_sb[j][:, tt * 512:(tt + 1) * 512],
                        start=(j == 0), stop=(j == NJP - 1))
                nc.vector.tensor_scalar_add(
                    dst[oc][:, tt * 512:(tt + 1) * 512], psq[:],
                    b_sb[:, oc:oc + 1])

            def emit_outproj_tcc(q, tcc, skip_hp=None):
                tl = tcc % 4
                pso = pg_pool.tile([128, 512], F32, tag="pg", name="pg")
                first = True
                for src_i, ot_sb in ((0, ot_a), (1, ot_b)):
                    for hp in range(NHP):
                        nc.tensor.matmul(
                            pso[:],
                            lhsT=ot_sb[q][:, hp, tl * 128:(tl + 1) * 128],
                            rhs=wo_sb[:, src_i, hp, :],
                            start=first,
                            stop=(src_i == 1 and hp == NHP - 1))
                        first = False
                ost = outstage.tile([128, 512], F32, tag="ost", name="ost")
                nc.vector.tensor_add(ost[:], pso[:], bo_sb[:])
                nc.sync.dma_start(
                    out=out_ext[tcc * 128:(tcc + 1) * 128, :],
                    in_=ost[:])

            def emit_drain(key):
                kind = key[0]
                if kind == "v":
                    emit_v(key[1])
                elif kind in ("k", "q"):
                    emit_kq(key[1], key[2], kind)
                else:
                    emit_outproj_tcc(key[1], key[2])

            # ---- warm the ACT exp table during the DMA window: walrus
            # attaches the ~2.7us ACT_TABLE_LOAD to the first ACTIVATE ----
            warm = persist.tile([128, 8], F32, tag="warm", name="warm")
            nc.vector.memset(warm[:], 0.0)
            nc.scalar.activation(warm[:], warm[:],
                                 mybir.ActivationFunctionType.Exp)
            # ---- keep HAM un-throttled through the DMA head: ~11.5us of
            # junk matmuls so the prefix K00/Q00 run at 2.4GHz, not 1.2 ----
            junk_w = persist.tile([128, 128], BF16, tag="junkw", name="junkw")
            junk_x = persist.tile([128, 512], BF16, tag="junkx", name="junkx")
            nc.vector.memset(junk_w[:], 0.0)
            nc.vector.memset(junk_x[:], 0.0)
            warm_ps = pg_pool.tile([128, 512], F32, tag="pg", name="pg")
            for _ in range(N_WARM_MM):
                nc.tensor.matmul(warm_ps[:], lhsT=junk_w[:], rhs=junk_x[:],
                                 start=True, stop=True)

            # ---- prefix: just K(oc0,tt0) + Q(oc0,tq0) so scores start
            # the moment their DMAs land; V pieces are unit-0 drains ----
            emit_kq(0, 0, "k")
            emit_kq(0, 0, "q")
            for tcc in range(PREFIX_V):
                emit_v(tcc)

            # ---- main units ----
            sched = _drain_schedule()

            po2_of_unit = {}

            def alloc_po2(ui):
                po2_of_unit[ui] = [
                    pa_pool.tile([128, 512], F32, tag="pa", name="pa")
                    for _ in range(2)]
                return po2_of_unit[ui]

            e_tiles = {}   # (ui, kc) -> sbuf exp tile

            def emit_scores(ui, kc):
                hp, tq = UNITS[ui]
                ps = ps_pool.tile([128, 1024], F32, tag="ps", name="ps")
                for h2 in (0, 1):
                    nc.tensor.matmul(
                        ps[:, h2 * 512:(h2 + 1) * 512],
                        lhsT=kt_sb[hp][h2 * 64:(h2 + 1) * 64, kc * 128:(kc + 1) * 128],
                        rhs=qt_sb[hp][h2 * 64:(h2 + 1) * 64, tq * 512:(tq + 1) * 512],
                        start=True, stop=True,
                        tile_position=(h2 * 64, 0))
                e_t = epool.tile([128, 1024], BF16, tag="e", name="e")
                nc.scalar.activation(e_t[:], ps[:],
                                     mybir.ActivationFunctionType.Exp)
                e_tiles[(ui, kc)] = e_t

            def emit_attnv(ui, kc):
                po2 = po2_of_unit[ui]
                e_t = e_tiles.pop((ui, kc))
                for h2 in (0, 1):
                    nc.tensor.matmul(
                        po2[h2][:],
                        lhsT=vh_sb[kc][:, 2 * UNITS[ui][0] + h2, :],
                        rhs=e_t[:, h2 * 512:(h2 + 1) * 512],
                        start=(kc == 0), stop=(kc == NTC - 1))

            def emit_normalize_ag(ui, halves=1):
                """Evacuate attnV accumulators, normalize (single fused
                divide), exchange. halves=2 pipelines the last unit's tail:
                each 256-column half normalizes and AllGathers separately so
                the first readback lands ~4us earlier."""
                hp, tq = UNITS[ui]
                po2 = po2_of_unit.pop(ui)
                om = evac.tile([128, 512], F32, tag="om", name="om")
                pk = evac.tile([128, 512], F32, tag="pk", name="pk")
                rr = evac.tile([128, 512], F32, tag="rr", name="rr")
                hw = 512 // halves
                for hf in range(halves):
                    sl = slice(hf * hw, (hf + 1) * hw)
                    for h2 in (0, 1):
                        nc.vector.tensor_copy(
                            pk[h2 * 64:(h2 + 1) * 64, sl], po2[h2][64:128, sl])
                        nc.vector.tensor_copy(
                            om[h2 * 64:(h2 + 1) * 64, sl], po2[h2][0:64, sl])
                    nc.vector.reciprocal(rr[:, sl], pk[:, sl])
                    nc.vector.tensor_mul(ot_a[tq][:, hp, sl], om[:, sl],
                                         rr[:, sl])
                    oT_in = dram.tile([128, hw], BF16, name=f"oTi{tq}_{hp}_{hf}")
                    oT_out = dram.tile([2, 128, hw], BF16,
                                       name=f"oTo{tq}_{hp}_{hf}")
                    nc.sync.dma_start(out=oT_in[:], in_=ot_a[tq][:, hp, sl])
                    nc.gpsimd.collective_compute(
                        "AllGather",
                        mybir.AluOpType.bypass,
                        ins=[oT_in.opt()],
                        outs=[oT_out.opt()],
                        replica_groups=[[0, 1], [2, 3], [4, 5], [6, 7]],
                    )
                    nc.sync.dma_start(out=ot_a[tq][:, hp, sl], in_=oT_out[0])
                    nc.sync.dma_start(out=ot_b[tq][:, hp, sl], in_=oT_out[1])

            # flat slot -> attnV groups emitted there (unit0 lags GLAG
            # slots so V-projection drains fit; units >=1 lag LAG1 so
            # normalize+AG land early)
            attn_slot = {}
            for u in range(len(UNITS)):
                for g in range(NSLOT):
                    attn_slot.setdefault(_attn_slot_of(u, g), []).append((u, g))

            for ui in range(len(UNITS)):
                alloc_po2(ui)
                for s in range(NSLOT):
                    emit_scores(ui, 2 * s)
                    emit_scores(ui, 2 * s + 1)
                    done_units = []
                    for vu, vg in attn_slot.get(ui * NSLOT + s, []):
                        emit_attnv(vu, 2 * vg)
                        emit_attnv(vu, 2 * vg + 1)
                        if vg == NSLOT - 1:
                            done_units.append(vu)
                    for key in sched[ui][s]:
                        emit_drain(key)
                    # normalize last: its ~6us DVE chain (copies + iterative
                    # divide) must not head-of-line block the drains'
                    # bias-adds that gate upcoming scores.
                    for vu in done_units:
                        emit_normalize_ag(vu)

            # ---- tail: last unit's spilled attnV groups, its normalize/AG,
            # then quarter-3 out-proj split around the final AllGather ----
            last = len(UNITS) - 1
            for vu, vg in [pair for f, pairs in attn_slot.items()
                           if f >= len(UNITS) * NSLOT for pair in pairs]:
                emit_attnv(vu, 2 * vg)
                emit_attnv(vu, 2 * vg + 1)
            emit_normalize_ag(last, halves=2)

            # phase A: hp0..2 contributions for quarter 3 accumulate while
            # AG(q3,hp3) is in flight; phase B finishes with the hp3 matmuls.
            # hp emission follows AG readiness ((0,3) early, (2,3) then
            # (1,3) late) so the in-order PE queue never blocks.
            # two waves matched to the AllGather halves: wave hf covers
            # tcc 12+2hf, 13+2hf. Phase A (hp0,2,1 in AG-readiness order)
            # accumulates while AG halves are in flight; hp3 finishes after
            # the half's readback. pg ring serializes the waves naturally.
            for hf in range(2):
                ps2 = [pg_pool.tile([128, 512], F32, tag="pg", name="pg")
                       for _ in range(2)]
                for hi, hp in enumerate((0, 2, 1)):
                    for i in range(2):
                        tl = 2 * hf + i
                        for src_i, ot_sb in ((0, ot_a), (1, ot_b)):
                            nc.tensor.matmul(
                                ps2[i][:],
                                lhsT=ot_sb[3][:, hp, tl * 128:(tl + 1) * 128],
                                rhs=wo_sb[:, src_i, hp, :],
                                start=(hi == 0 and src_i == 0), stop=False)
                for j, (src_i, ot_sb) in enumerate(((0, ot_a), (1, ot_b))):
                    for i in range(2):
                        tl = 2 * hf + i
                        nc.tensor.matmul(
                            ps2[i][:],
                            lhsT=ot_sb[3][:, NHP - 1, tl * 128:(tl + 1) * 128],
                            rhs=wo_sb[:, src_i, NHP - 1, :],
                            start=False, stop=(j == 1))
                for i in range(2):
                    tcc = 3 * 4 + 2 * hf + i
                    ost = outstage.tile([128, 512], F32, tag="ost", name="ost")
                    nc.vector.tensor_add(ost[:], ps2[i][:], bo_sb[:])
                    nc.sync.dma_start(
                        out=out_ext[tcc * 128:(tcc + 1) * 128, :],
                        in_=ost[:])

    if split_sync:
        _split_sync_commands(nc)
    return nc


_NC_CACHE = {}


def _get_nc():
    if "nc" not in _NC_CACHE:
        _NC_CACHE["nc"] = build_nc()
    return _NC_CACHE["nc"]


def _prep_core_inputs(x, Wq, bq, Wk, bk, Wv, bv, Wo, bo):
    """Host-side sharding + layout. Returns in_maps list (8 cores)."""
    x = np.asarray(x, np.float32)
    s = 1.0 / np.sqrt(np.float32(DK))
    Wq_s, bq_s = np.asarray(Wq, np.float32) * s, np.asarray(bq, np.float32) * s
    Wk_f, bk_f = np.asarray(Wk, np.float32), np.asarray(bk, np.float32)
    Wv_f, bv_f = np.asarray(Wv, np.float32), np.asarray(bv, np.float32)
    Wo_f, bo_f = np.asarray(Wo, np.float32), np.asarray(bo, np.float32)

    in_maps = []
    for c in range(N_CORES):
        b, g = c // 2, c % 2
        cols = slice(g * 512, (g + 1) * 512)
        wq_g, bq_g = Wq_s[:, cols], bq_s[cols]
        wk_g, bk_g = Wk_f[:, cols], bk_f[cols]
        wv_g, bv_g = Wv_f[:, cols], bv_f[cols]

        xt_dev = np.ascontiguousarray(x[b].T).reshape(NJP, 128, T).astype(NPBF16)

        def wqk_dev(w):
            # [jp, r, oc, c] -> [oc, r, jp, c] (oc-major for split DMA)
            return np.ascontiguousarray(
                w.reshape(NJP, 128, NOC, 128).transpose(2, 1, 0, 3)).astype(NPBF16)

        wv_dev = np.ascontiguousarray(
            wv_g.reshape(NJP, 128, 512).transpose(1, 0, 2)).astype(NPBF16)

        # Wo rows regrouped to the on-device O^T layout, columns = this
        # core's output half (nb = core parity g):
        # rows [src group, hp, h2, 64] -> partitions h2*64+r, free [src, hp, col]
        wo_dev = (Wo_f[:, cols].reshape(2, NHP, 2, 64, 512)
                  .transpose(2, 3, 0, 1, 4)          # [h2, r, src, hp, col]
                  .reshape(128, 2, NHP, 512)).astype(NPBF16)
        bo_dev = bo_f[cols].reshape(1, 512)

        in_maps.append({
            "xt": xt_dev,
            "wq": wqk_dev(wq_g), "wk": wqk_dev(wk_g), "wv": wv_dev,
            "wo": wo_dev,
            "bq": np.ascontiguousarray(bq_g.reshape(NOC, 128).T),
            "bk": np.ascontiguousarray(bk_g.reshape(NOC, 128).T),
            "bv": bv_g.reshape(1, 512),
            "bo": bo_dev,
        })
    return in_maps


def kernel(x, Wq, bq, Wk, bk, Wv, bv, Wo, bo, _trace=False):
    nc = _get_nc()
    in_maps = _prep_core_inputs(x, Wq, bq, Wk, bk, Wv, bv, Wo, bo)
    res = run_bass_kernel_spmd(nc, in_maps, core_ids=list(range(N_CORES)),
                               trace=_trace)
    out = np.empty((B, T, D), np.float32)
    for b in range(B):
        for g in range(2):
            out[b][:, g * 512:(g + 1) * 512] = res.results[2 * b + g]["out"]
    if _trace:
        kernel.last_result = res
    return out
